# revision 40
# baseline (speedup 1.0000x reference)
"""AutoregressiveMlpMixer forward on 8 Trainium2 NeuronCores (Bass/Tile).

Strategy (v2)
- Pure data parallelism: 64 batch items -> 8 per core, weights replicated.
- The reverse cumsum over tokens is folded into tok_w1 on the host
  (suffix-sum then matmul == matmul with prefix-cumsum'd weights).
- LN2 / final-LN affine params are folded into the following matmul weights
  on the host. tok_b2 is dropped exactly (constant along the LN2 axis).
- All matmul operands are fp16 (same 1 cyc/row PE rate as f32r, half the
  HBM traffic and SBUF footprint, ~2e-3 rel err). PSUM stays fp32.
- Inter-block state X is kept TRANSPOSED in item-pair tiles X2[pair][ct] =
  [128(c), 2*256(tok)]; LN1 re-transposes on the PE.
- Channel-MLP runs per group of G=4 items (weights stream once per block):
  E produces all 24 gelu tiles (HG, SBUF-resident), then F accumulates per
  (mt-half, c-tile) in a single rotating PSUM bank and folds into X2.
  PSUM is split into dedicated AD (LN/token-mix, 4 banks) and EF (4 banks)
  pools so the two phases never starve each other.
- Channel weights: ch_w2 half-block = ONE 18KB/partition DMA (SWDGE, Pool
  queue, re-issued right after the F half that releases it); ch_w1 streams
  in 4-mt chunks on SP. ~170 DMAs total (vs ~1600 naively).
- LN rsqrt = group-batched Newton iterations on DVE (no act-table
  switches; the Act engine only ever holds the gelu table).
- The head reuses the dead HG/W2S tiles so its DMAs/LN overlap the last
  F pass instead of waiting for the mixer pools to release.
"""

import sys

sys.path.insert(0, "/opt/trn_rl_repo")

import numpy as np

import concourse.bass as bass
import concourse.tile as tile
from concourse import bacc, masks, mybir

f32 = mybir.dt.float32
f32r = mybir.dt.float32r
f16 = mybir.dt.float16
i32 = mybir.dt.int32
AF = mybir.ActivationFunctionType
ALU = mybir.AluOpType

# Model dims (hardcoded per problem spec)
B, CIN, H, W = 64, 2, 32, 32
N = 256          # tokens
C = 768          # hidden dim
TOK = 512        # tokens_mlp_dim
CH = 3072        # channels_mlp_dim
L = 8            # blocks
K = 2048         # num_classes
EPS = 1e-5

NCORES = 8
IPC = B // NCORES    # items per core = 8
NT = N // 128        # 2 token tiles per item
CT = C // 128        # 6 channel tiles
MT = CH // 128       # 24 channel-mlp tiles
TT = TOK // 128      # 4 token-mlp tiles
CC = (512, 256)      # channel free-dim chunks for 768
CCO = (0, 512)
G = 4                # items per channel-MLP weight pass
WCH = 4              # mt per w1 stream chunk
MAGIC = 0x5F3759DF


def _batched_rsqrt(nc, pool, mv, n, magic16, tag):
    """mv: [128, n, 2] f32 (mean, var) chains. Returns rstd tile [128, n].

    Newton on DVE: no activation-table traffic."""
    vv = pool.tile([128, n], f32, tag=f"{tag}_v", bufs=4, name="vv")
    nc.vector.tensor_scalar_add(vv, mv[:, :, 1], float(EPS))
    iv = pool.tile([128, n], i32, tag=f"{tag}_i", bufs=4, name="iv")
    nc.vector.tensor_scalar(iv, vv.bitcast(i32), 1, None,
                            ALU.logical_shift_right)
    nc.vector.tensor_tensor(iv, magic16[:, :n], iv, ALU.subtract)
    y = iv.bitcast(f32)
    t = pool.tile([128, n], f32, tag=f"{tag}_t", bufs=4, name="tt")
    for _ in range(2):
        nc.vector.tensor_mul(t, y, y)
        nc.vector.tensor_mul(t, t, vv)
        nc.vector.tensor_scalar(t, t, -0.5, 1.5, ALU.mult, ALU.add)
        nc.vector.tensor_mul(y, y, t)
    return y


def build(items=IPC, blocks=L, has_g1=False, has_b1=False):
    """Build the SPMD program for one core processing `items` batch items."""
    nc = bacc.Bacc("TRN2", target_bir_lowering=False, debug=False)

    bl = max(blocks, 1)
    n_groups = (items + G - 1) // G
    npairs = (items + 1) // 2

    # ---- DRAM tensors (names = in_map keys) ----
    pt = nc.dram_tensor("pt", [9, items * N], f32r, kind="ExternalInput")
    wq = nc.dram_tensor("wq", [9, C], f32r, kind="ExternalInput")
    tokw1c = nc.dram_tensor("tokw1c", [bl, NT, 128, TOK], f16,
                            kind="ExternalInput")
    tokw2 = nc.dram_tensor("tokw2", [bl, TT, 128, N], f16,
                           kind="ExternalInput")
    tokb1 = nc.dram_tensor("tokb1", [bl, 128, TT], f32, kind="ExternalInput")
    vb1 = nc.dram_tensor("vb1", [bl, 128, MT], f32, kind="ExternalInput")
    chb2c = nc.dram_tensor("chb2c", [bl, 128, CT], f32, kind="ExternalInput")
    # ch_w1 (g2-folded): chunks of WCH mt: [bl, MT/WCH, 128, WCH*768]
    w1gd = nc.dram_tensor("w1gd", [bl, MT // WCH, 128, WCH * C], f16,
                          kind="ExternalInput")
    # ch_w2: whole block, partition-major: [bl, 128, MT, 768]
    w2d = nc.dram_tensor("w2d", [bl, 128, MT, C], f16, kind="ExternalInput")
    headwg = nc.dram_tensor("headwg", [CT, 128, K], f16, kind="ExternalInput")
    headb = nc.dram_tensor("headb", [1, K], f16, kind="ExternalInput")
    ln1g = nc.dram_tensor("ln1g", [bl, C], f32, kind="ExternalInput")
    ln1b = nc.dram_tensor("ln1b", [bl, C], f32, kind="ExternalInput")
    out = nc.dram_tensor("out", [items, K], f32, kind="ExternalOutput")

    with tile.TileContext(nc) as tc:
        with tc.tile_pool(name="const", bufs=1) as const, \
             tc.tile_pool(name="xstate", bufs=1) as xstate:
            magic16 = const.tile([128, 16], i32, name="magic16")
            nc.vector.memset(magic16, MAGIC)
            ident = const.tile([128, 128], f32, name="ident")
            masks.make_identity(nc, ident)
            identb = const.tile([128, 128], f16, name="identb")
            nc.vector.tensor_copy(identb, ident)

            # persistent state, TRANSPOSED, item-PAIR tiles:
            # X2[p][ct] = [128(c), 512] = items (2p, 2p+1)
            X2 = [[xstate.tile([128, 2 * N], f16, name=f"x_{p}_{ct}")
                   for ct in range(CT)] for p in range(npairs)]
            # channel-MLP gelu activations, SBUF-resident for a whole group
            HG = [xstate.tile([128, G * N], f16, name=f"hg_{mt}")
                  for mt in range(MT)]
            # ch_w2 for the current block in two half-block tiles
            # [128(m within mt), mt-half, 768(c)]
            MH = MT // 2
            W2S = [xstate.tile([128, MH, C], f16, name=f"w2s_{hf}")
                   for hf in range(2)]

            # ---------------- stem (writes X2 transposed) ----------------
            with tc.tile_pool(name="stem", bufs=1) as stem, \
                 tc.tile_pool(name="ps_stem", bufs=4, space="PSUM") as ps_stem:
                ptt = stem.tile([9, items * N], f32r)
                nc.sync.dma_start(out=ptt, in_=pt[:, :])
                wqt = stem.tile([9, C], f32r)
                nc.sync.dma_start(out=wqt, in_=wq[:, :])
                for p in range(npairs):
                    for ct in range(CT):
                        pss = ps_stem.tile([128, 512], f32, tag="pss",
                                           name="pss")
                        nc.tensor.matmul(pss,
                                         wqt[:, ct * 128:(ct + 1) * 128],
                                         ptt[:, p * 512:(p + 1) * 512],
                                         start=True, stop=True)
                        if ct % 2 == 0:
                            nc.scalar.activation(out=X2[p][ct], in_=pss,
                                                 func=AF.Copy)
                        else:
                            nc.vector.tensor_copy(X2[p][ct], pss)

            # ---------------- mixer blocks ----------------
            with tc.tile_pool(name="tokw", bufs=1) as tokwp, \
                 tc.tile_pool(name="lnp", bufs=4) as lnp, \
                 tc.tile_pool(name="acts", bufs=1) as acts, \
                 tc.tile_pool(name="wstream", bufs=2) as wstream, \
                 tc.tile_pool(name="ps_ad", bufs=2, space="PSUM") as ps_ad, \
                 tc.tile_pool(name="ps_ef", bufs=2, space="PSUM") as ps_ef:

                blk_w = {}

                def emit_tok_weights(l):
                    w = {}
                    w1c_t = tokwp.tile([128, NT, TOK], f16, tag="w1c",
                                       bufs=1, name="w1c")
                    nc.sync.dma_start(out=w1c_t,
                                      in_=tokw1c[l].rearrange("k p t -> p k t"))
                    w2_t = tokwp.tile([128, TT, N], f16, tag="w2", bufs=1,
                                      name="w2")
                    nc.sync.dma_start(out=w2_t,
                                      in_=tokw2[l].rearrange("k p n -> p k n"))
                    b1_t = tokwp.tile([128, TT], f32, tag="b1", bufs=2,
                                      name="b1")
                    nc.sync.dma_start(out=b1_t, in_=tokb1[l])
                    vb1_t = tokwp.tile([128, MT], f32, tag="vb1", bufs=2,
                                       name="vb1")
                    nc.sync.dma_start(out=vb1_t, in_=vb1[l])
                    chb2_t = tokwp.tile([128, CT], f32, tag="chb2", bufs=2,
                                        name="chb2")
                    nc.sync.dma_start(out=chb2_t, in_=chb2c[l])
                    w.update(w1c=w1c_t, w2=w2_t, b1=b1_t, vb1=vb1_t,
                             chb2=chb2_t)
                    if has_g1:
                        g1_t = tokwp.tile([128, C], f32, tag="g1", bufs=2,
                                          name="g1")
                        nc.sync.dma_start(
                            out=g1_t,
                            in_=ln1g.ap()[l:l + 1, :].partition_broadcast(128))
                        w["g1"] = g1_t
                    if has_b1:
                        b1v_t = tokwp.tile([128, C], f32, tag="b1v", bufs=2,
                                           name="b1v")
                        nc.sync.dma_start(
                            out=b1v_t,
                            in_=ln1b.ap()[l:l + 1, :].partition_broadcast(128))
                        w["b1v"] = b1v_t
                    return w

                def emit_AD(l, g, hb=G):
                    """token-mix + LN stages for group g of block l -> Zt.

                    hb = LN-batch granularity in items. Whole-group batches
                    interleave best in steady state; the first (pipeline
                    fill) step uses hb=2 so E can start after 4 LN chains."""
                    if l not in blk_w:
                        blk_w[l] = emit_tok_weights(l)
                    w1c_t, w2_t, b1_t = (blk_w[l][k] for k in ("w1c", "w2", "b1"))
                    g1_t = blk_w[l].get("g1")
                    b1v_t = blk_w[l].get("b1v")
                    gitems = list(range(g * G, min((g + 1) * G, items)))
                    Zt = [acts.tile([128, G * N], f16, tag=f"zt_{kc}",
                                    bufs=2, name=f"zt_{kc}")
                          for kc in range(CT)]
                    for h0 in range(0, len(gitems), hb):
                        hitems = gitems[h0:h0 + hb]
                        chains = [(h0 + k, i, t)
                                  for k, i in enumerate(hitems)
                                  for t in range(NT)]
                        nch = len(chains)
                        # ---- A: transposes + copies + stats ----
                        xn = {}
                        mv1 = lnp.tile([128, nch, 2], f32, tag="mv1", bufs=2,
                                       name="mv1")
                        for j, (i2, i, t) in enumerate(chains):
                            ptr = ps_ad.tile([128, C], f16, tag="ptr",
                                             name="ptrA")
                            for cc in range(CT):
                                nc.tensor.transpose(
                                    ptr[:, cc * 128:(cc + 1) * 128],
                                    X2[i // 2][cc][:, (i % 2) * N + t * 128:
                                                   (i % 2) * N + (t + 1) * 128],
                                    identb)
                            xt = lnp.tile([128, C], f16, tag="xn", bufs=8,
                                          name="xn")
                            nc.vector.tensor_copy(xt, ptr)
                            xn[(i2, t)] = xt
                            st = lnp.tile([128, 3, 6], f32, tag="st1", bufs=8,
                                          name="st")
                            pgg = ptr.rearrange("p (s q) -> p s q", q=256)
                            for s in range(3):
                                nc.vector.bn_stats(out=st[:, s, :],
                                                   in_=pgg[:, s, :])
                            nc.vector.bn_aggr(out=mv1[:, j, :], in_=st)
                        rstd1 = _batched_rsqrt(nc, lnp, mv1, nch, magic16,
                                               "r1")
                        # ---- LN1 apply ----
                        Y = {}
                        for j, (i2, i, t) in enumerate(chains):
                            yt = lnp.tile([128, C], f16, tag="yt", bufs=8,
                                          name="yt")
                            nc.vector.tensor_scalar(
                                out=yt, in0=xn[(i2, t)],
                                scalar1=mv1[:, j, 0:1],
                                scalar2=rstd1[:, j:j + 1],
                                op0=ALU.subtract, op1=ALU.mult)
                            if has_g1:
                                nc.vector.tensor_mul(yt, yt, g1_t)
                            if has_b1:
                                nc.vector.tensor_add(yt, yt, b1v_t)
                            Y[(i2, t)] = yt
                        # ---- B + C per item (B: y1 = gelu(w1cum^T @ Y + b1);
                        #      C: y2 = w2^T @ y1 with LN2 stats from PSUM) ----
                        y2 = {}
                        mv2 = lnp.tile([128, nch, 2], f32, tag="mv2", bufs=2,
                                       name="mv2")
                        for k, i in enumerate(hitems):
                            i2 = h0 + k
                            y1 = []
                            for mt in range(TT):
                                yg = lnp.tile([128, C], f16, tag="y1", bufs=8,
                                              name="yg")
                                for cw, co in zip(CC, CCO):
                                    pb = ps_ad.tile([128, 512], f32, tag="mm",
                                                    name="pb")
                                    for kk in range(NT):
                                        nc.tensor.matmul(
                                            pb[:, :cw],
                                            w1c_t[:, kk,
                                                  mt * 128:(mt + 1) * 128],
                                            Y[(i2, kk)][:, co:co + cw],
                                            start=(kk == 0),
                                            stop=(kk == NT - 1))
                                    nc.scalar.activation(
                                        out=yg[:, co:co + cw], in_=pb[:, :cw],
                                        func=AF.Gelu, bias=b1_t[:, mt:mt + 1],
                                        scale=1.0)
                                y1.append(yg)
                            for t in range(NT):
                                j = k * NT + t
                                y2t = lnp.tile([128, C], f16, tag="y2",
                                               bufs=8, name="y2t")
                                st = lnp.tile([128, 3, 6], f32, tag="st2",
                                              bufs=8, name="st2")
                                for ci, (cw, co) in enumerate(zip(CC, CCO)):
                                    pc = ps_ad.tile([128, 512], f32, tag="mm",
                                                    name="pc")
                                    for kk in range(TT):
                                        nc.tensor.matmul(
                                            pc[:, :cw],
                                            w2_t[:, kk, t * 128:(t + 1) * 128],
                                            y1[kk][:, co:co + cw],
                                            start=(kk == 0),
                                            stop=(kk == TT - 1))
                                    nc.scalar.activation(
                                        out=y2t[:, co:co + cw],
                                        in_=pc[:, :cw], func=AF.Copy)
                                    pg = pc[:, :cw].rearrange(
                                        "p (s q) -> p s q", q=256)
                                    for s in range(cw // 256):
                                        nc.vector.bn_stats(
                                            out=st[:, 2 * ci + s, :],
                                            in_=pg[:, s, :])
                                nc.vector.bn_aggr(out=mv2[:, j, :], in_=st)
                                y2[(i2, t)] = y2t
                        rstd2 = _batched_rsqrt(nc, lnp, mv2, nch, magic16,
                                               "r2")
                        # ---- LN2 apply + transpose into Zt ----
                        for j, (i2, i, t) in enumerate(chains):
                            zn = lnp.tile([128, C], f16, tag="zn", bufs=2,
                                          name="zn")
                            nc.vector.tensor_scalar(
                                out=zn, in0=y2[(i2, t)],
                                scalar1=mv2[:, j, 0:1],
                                scalar2=rstd2[:, j:j + 1],
                                op0=ALU.subtract, op1=ALU.mult)
                            ptr = ps_ad.tile([128, C], f16, tag="ptr",
                                             name="ptrT")
                            for cc in range(CT):
                                nc.tensor.transpose(
                                    ptr[:, cc * 128:(cc + 1) * 128],
                                    zn[:, cc * 128:(cc + 1) * 128],
                                    identb)
                            for cc in range(CT):
                                nc.vector.tensor_copy(
                                    Zt[cc][:, i2 * N + t * 128:
                                           i2 * N + (t + 1) * 128],
                                    ptr[:, cc * 128:(cc + 1) * 128])
                    return Zt

                def emit_EF(l, g, Zt):
                    """channel-MLP for group g of block l.

                    E: all MT gelu tiles into SBUF (HG); F: per mt-half, per
                    output c-tile, accumulate MH k-tiles in one PSUM bank,
                    fold to X2 (bias add on half 0, accumulate on half 1).
                    The next block's W2S half-tiles are re-loaded (DVE queue)
                    right after the last F reads that release them."""
                    vb1_t = blk_w[l]["vb1"]
                    chb2_t = blk_w[l]["chb2"]
                    gitems = list(range(g * G, min((g + 1) * G, items)))
                    nw = len(gitems) * N
                    nh = (nw + 511) // 512
                    last_g = g == n_groups - 1
                    for ch in range(MT // WCH):
                        w1g_t = wstream.tile([128, WCH, C], f16, tag="w1g",
                                             name="w1g_t")
                        nc.sync.dma_start(out=w1g_t, in_=w1gd[l, ch].rearrange(
                            "p (j c) -> p j c", j=WCH))
                        for jj in range(WCH):
                            mt = ch * WCH + jj
                            for h in range(nh):
                                hw_ = min(512, nw - h * 512)
                                pe = ps_ef.tile([128, 512], f32, tag="pe",
                                                name="pe")
                                for kc in range(CT):
                                    nc.tensor.matmul(
                                        pe[:, :hw_],
                                        w1g_t[:, jj, kc * 128:(kc + 1) * 128],
                                        Zt[kc][:, h * 512:h * 512 + hw_],
                                        start=(kc == 0), stop=(kc == CT - 1))
                                nc.scalar.activation(
                                    out=HG[mt][:, h * 512:h * 512 + hw_],
                                    in_=pe[:, :hw_], func=AF.Gelu,
                                    bias=vb1_t[:, mt:mt + 1], scale=1.0)
                    for hf in range(2):
                        for ct in range(CT):
                            for h in range(nh):
                                hw_ = min(512, nw - h * 512)
                                pf = ps_ef.tile([128, 512], f32, tag="pf",
                                                name="pf")
                                for mj in range(MH):
                                    mt = hf * MH + mj
                                    nc.tensor.matmul(
                                        pf[:, :hw_],
                                        W2S[hf][:, mj, ct * 128:(ct + 1) * 128],
                                        HG[mt][:, h * 512:h * 512 + hw_],
                                        start=(mj == 0), stop=(mj == MH - 1))
                                p = g * (G // 2) + h
                                if hf == 0:
                                    # fold with ch_b2 bias (Act, table-free)
                                    nc.scalar.activation(
                                        out=X2[p][ct][:, :hw_],
                                        in_=pf[:, :hw_], func=AF.Identity,
                                        bias=chb2_t[:, ct:ct + 1], scale=1.0)
                                else:
                                    nc.vector.tensor_tensor(
                                        X2[p][ct][:, :hw_], X2[p][ct][:, :hw_],
                                        pf[:, :hw_], ALU.add)
                        if last_g and l + 1 < blocks:
                            # prefetch next block's ch_w2 half (DVE queue; its
                            # WAR waits were just satisfied by this F half)
                            nc.gpsimd.dma_start(
                                out=W2S[hf],
                                in_=w2d[l + 1, :, hf * MH:(hf + 1) * MH, :])

                # initial ch_w2 load for block 0 (SP queue: in-order behind
                # the stem/tok DMAs so it cannot delay their transfers)
                for hf in range(2):
                    nc.sync.dma_start(
                        out=W2S[hf], in_=w2d[0, :, hf * MH:(hf + 1) * MH, :])
                # software-pipelined emission: A-D of step s+1 lands before
                # E/F of step s so the scheduler can fill LN-latency bubbles.
                seq = [(l, g) for l in range(blocks) for g in range(n_groups)]
                zts = {}
                if seq:
                    zts[seq[0]] = emit_AD(*seq[0])
                for idx, key in enumerate(seq):
                    if idx + 1 < len(seq):
                        nkey = seq[idx + 1]
                        zts[nkey] = emit_AD(*nkey)
                    emit_EF(*key, zts.pop(key))

            # ---------------- final LN + token-mean + head ----------------
            with tc.tile_pool(name="headp", bufs=1) as headp, \
                 tc.tile_pool(name="lnf", bufs=4) as lnf, \
                 tc.tile_pool(name="ps_h", bufs=2, space="PSUM") as ps_h:
                invn_f = headp.tile([128, 2], f32)
                nc.vector.memset(invn_f, 1.0 / N)
                invn_col = headp.tile([128, 2], f16)
                nc.vector.tensor_copy(invn_col, invn_f)
                ones8_f = headp.tile([1, items], f32)
                nc.vector.memset(ones8_f, 1.0)
                ones8 = headp.tile([1, items], f16)
                nc.vector.tensor_copy(ones8, ones8_f)
                xmall = headp.tile([128, CT, items], f16)
                # prefetch head weights into the (now dead) W2S tiles: their
                # SBUF is persistent, so the DMAs only wait on the last F
                # reads instead of on the whole mixer-pool release.
                hb_t = headp.tile([1, K], f16)
                nc.sync.dma_start(out=hb_t, in_=headb[:, :])
                hw_ts = []
                for jc in range(K // 512):
                    flat = W2S[jc // 2].rearrange("p a b -> p (a b)")
                    hwv = flat[:, (jc % 2) * CT * 512:
                               (jc % 2 + 1) * CT * 512].rearrange(
                        "p (a b) -> p a b", a=CT)
                    nc.sync.dma_start(
                        out=hwv,
                        in_=headwg.ap()[:, :, jc * 512:(jc + 1) * 512]
                        .rearrange("c p k -> p c k"))
                    hw_ts.append(hwv)
                chains = [(i, t) for i in range(items) for t in range(NT)]
                mvf = lnf.tile([128, items * NT, 2], f32, tag="mvf", bufs=1,
                               name="mvf")
                xf = {}
                for j, (i, t) in enumerate(chains):
                    ptr = ps_h.tile([128, C], f16, tag="pth", bufs=4,
                                    name="ptrH")
                    for cc in range(CT):
                        nc.tensor.transpose(
                            ptr[:, cc * 128:(cc + 1) * 128],
                            X2[i // 2][cc][:, (i % 2) * N + t * 128:
                                           (i % 2) * N + (t + 1) * 128],
                            identb)
                    # reuse the dead HG tiles as xf storage (persistent SBUF)
                    xt = HG[j][:, :C]
                    nc.scalar.activation(out=xt, in_=ptr, func=AF.Copy)
                    xf[(i, t)] = xt
                    st = lnf.tile([128, 3, 6], f32, tag="stf", bufs=8,
                                  name="stf")
                    pgg = ptr.rearrange("p (s q) -> p s q", q=256)
                    for s in range(3):
                        nc.vector.bn_stats(out=st[:, s, :], in_=pgg[:, s, :])
                    nc.vector.bn_aggr(out=mvf[:, j, :], in_=st)
                rstdf = _batched_rsqrt(nc, lnf, mvf, items * NT, magic16,
                                       "rf")
                for i in range(items):
                    xh = []
                    for t in range(NT):
                        j = i * NT + t
                        xht = lnf.tile([128, C], f16, tag="xh", bufs=4,
                                       name="xht")
                        nc.vector.tensor_scalar(
                            out=xht, in0=xf[(i, t)],
                            scalar1=mvf[:, j, 0:1], scalar2=rstdf[:, j:j + 1],
                            op0=ALU.subtract, op1=ALU.mult)
                        xh.append(xht)
                    pxm = ps_h.tile([128, CT, 2], f32, tag="pxm", name="pxm")
                    for ct in range(CT):
                        for t in range(NT):
                            nc.tensor.matmul(pxm[:, ct, :],
                                             xh[t][:, ct * 128:(ct + 1) * 128],
                                             invn_col,
                                             start=(t == 0), stop=(t == NT - 1))
                    nc.vector.tensor_copy(xmall[:, :, i], pxm[:, :, 0])
                outsb = headp.tile([items, K], f32)
                for jc in range(K // 512):
                    hw_t = hw_ts[jc]
                    ph = ps_h.tile([items, 512], f32, tag="ph", name="ph")
                    for ct in range(CT):
                        nc.tensor.matmul(ph, xmall[:, ct, :items],
                                         hw_t[:, ct, :],
                                         start=(ct == 0), stop=False)
                    nc.tensor.matmul(ph, ones8, hb_t[:, jc * 512:(jc + 1) * 512],
                                     start=False, stop=True)
                    nc.scalar.activation(out=outsb[:, jc * 512:(jc + 1) * 512],
                                         in_=ph, func=AF.Copy)
                nc.sync.dma_start(out=out[:, :], in_=outsb)

    nc.compile()
    return nc


# ---------------------------------------------------------------------------
# host-side preprocessing
# ---------------------------------------------------------------------------

def prep_inputs(inputs, stem_w, stem_b, ln1_g, ln1_b, tok_w1, tok_b1, tok_w2,
                tok_b2, ln2_g, ln2_b, ch_w1, ch_b1, ch_w2, ch_b2, lnf_g, lnf_b,
                head_w, head_b, items=IPC, blocks=L):
    """Returns (shared_map, per_core_list, flags)."""
    f = np.float32
    b16 = np.float16
    inputs = np.asarray(inputs, f)
    # patches: (B, CIN, 16, 2, 16, 2) -> (B, n=256, q=8); +ones row -> (B,9,256)
    x = inputs.reshape(B, CIN, H // 2, 2, W // 2, 2).transpose(0, 2, 4, 1, 3, 5)
    x = x.reshape(B, N, CIN * 4)
    ptA = np.concatenate([x.transpose(0, 2, 1),
                          np.ones((B, 1, N), f)], axis=1)  # (B, 9, 256)

    wq = np.concatenate([np.asarray(stem_w, f).reshape(C, 8).T,
                         np.asarray(stem_b, f)[None, :]], axis=0)  # (9, C)

    blocks = max(blocks, 1)
    w1cum = np.cumsum(np.asarray(tok_w1, f), axis=1)[:blocks]        # (L, N, TOK)
    tokw1c = np.ascontiguousarray(w1cum.reshape(blocks, NT, 128, TOK))
    tokw2 = np.ascontiguousarray(np.asarray(tok_w2, f)[:blocks]
                                 .reshape(blocks, TT, 128, N))
    tokb1 = np.ascontiguousarray(np.asarray(tok_b1, f)[:blocks]
                                 .reshape(blocks, TT, 128).transpose(0, 2, 1))

    g2 = np.asarray(ln2_g, f)[:blocks]
    b2 = np.asarray(ln2_b, f)[:blocks]
    cw1 = np.asarray(ch_w1, f)[:blocks]
    w1g_full = g2[:, :, None] * cw1                                   # (L, C, CH)
    # per (mt): [128(c within ct), CT*128(m)] -> chunks of WCH mt
    w1g = (w1g_full.reshape(blocks, CT, 128, MT, 128)
           .transpose(0, 3, 2, 1, 4).reshape(blocks, MT, 128, C))
    w1gd = np.ascontiguousarray(
        w1g.reshape(blocks, MT // WCH, WCH, 128, C)
        .transpose(0, 1, 3, 2, 4).reshape(blocks, MT // WCH, 128, WCH * C))
    v = np.einsum("lc,lcm->lm", b2, cw1) + np.asarray(ch_b1, f)[:blocks]
    vb1 = np.ascontiguousarray(v.reshape(blocks, MT, 128).transpose(0, 2, 1))
    # ch_w2: [128(m within mt), MT, 768(c)]
    w2d = np.ascontiguousarray(np.asarray(ch_w2, f)[:blocks]
                               .reshape(blocks, MT, 128, C)
                               .transpose(0, 2, 1, 3))
    chb2c = np.ascontiguousarray(np.asarray(ch_b2, f)[:blocks]
                                 .reshape(blocks, CT, 128).transpose(0, 2, 1))

    gf = np.asarray(lnf_g, f)
    bf = np.asarray(lnf_b, f)
    hw = np.asarray(head_w, f)
    headwg = np.ascontiguousarray((gf[:, None] * hw).reshape(CT, 128, K))
    headb = (bf @ hw + np.asarray(head_b, f)).reshape(1, K).astype(f)

    ln1g = np.ascontiguousarray(np.asarray(ln1_g, f)[:blocks])
    ln1b = np.ascontiguousarray(np.asarray(ln1_b, f)[:blocks])
    has_g1 = not np.all(ln1g == 1.0)
    has_b1 = not np.all(ln1b == 0.0)

    shared = dict(wq=wq.astype(f), tokw1c=tokw1c.astype(b16),
                  tokw2=tokw2.astype(b16), tokb1=tokb1,
                  w1gd=w1gd.astype(b16), vb1=vb1, w2d=w2d.astype(b16),
                  chb2c=chb2c, headwg=headwg.astype(b16),
                  headb=headb.astype(b16), ln1g=ln1g, ln1b=ln1b)

    per_core = []
    for c in range(NCORES):
        sel = ptA[c * IPC:(c + 1) * IPC][:items]  # (items, 9, 256)
        ptc = np.ascontiguousarray(sel.transpose(1, 0, 2).reshape(9, items * N))
        per_core.append(dict(pt=ptc))
    return shared, per_core, dict(has_g1=has_g1, has_b1=has_b1)


_CACHE = {}


def kernel(**inputs):
    from concourse.bass_utils import run_bass_kernel_spmd
    shared, per_core, flags = prep_inputs(**inputs)
    key = (flags["has_g1"], flags["has_b1"])
    if key not in _CACHE:
        _CACHE[key] = build(has_g1=flags["has_g1"], has_b1=flags["has_b1"])
    nc = _CACHE[key]
    in_maps = [{**shared, **pc} for pc in per_core]
    res = run_bass_kernel_spmd(nc, in_maps, core_ids=list(range(NCORES)))
    outs = [r["out"] for r in res.results]
    return np.concatenate(outs, axis=0).astype(np.float32)
